# revision 2
# baseline (speedup 1.0000x reference)
"""GCN (3-layer GCNConv) on 8 TRN2 NeuronCores — single-NEFF implementation.

Schedule per layer:
  stage A: table shard = dinv * (H @ W) in bf16 (PE matmul + ACT scale), staged
           in one SBUF block, one DMA to the AllGather bounce, AllGather ->
           full [N_PAD, Fo] bf16 table in every core's DRAM.
  stage B: per 128-dst-node tile: one indirect-DMA gather of K=128*NB source
           rows; per batch of 128 edges one DVE scalar_tensor_tensor builds
           S[e, slot] = wgt_e @ dstlocal_e ((iota == dstl) * wgt), TensorE
           accumulates PSUM[feat, slot] += G_b^T @ S_b in f32; ACT drains
           relu(psum + bias) with features on partitions (so the next layer's
           H @ W needs no transpose).
  norm = dinv[src]*w*dinv[dst] folds into the table rows (src side) and the
  edge weight stream (dst side).  Node ids are permuted
  (pid = (c*128+p)*T + t) so each core's SBUF-staged shard all-gathers into a
  row-gatherable table.

Toolchain quirk: this walrus build allows ~zero semaphore waits on most
compute-instruction formats ("Too many sync wait commands").  After Tile
schedules the program, _split_waits() moves every on_wait onto its own
same-engine InstNoOp inserted immediately before the instruction — semantics
identical, and NoOps accept waits.
"""
import sys
import time

import numpy as np
import ml_dtypes

for p in ("/opt/trn_rl_repo",):
    if p not in sys.path:
        sys.path.insert(0, p)

BF16 = ml_dtypes.bfloat16
N_CORES = 8
R = 12544                 # dst rows per core
N_PAD = N_CORES * R       # 100352
T = R // 128              # 98 dst tiles per core
F = 128
N_CLS = 64
GT = 7                    # meta-load group: 7 tiles; 98 = 14*7
NG = T // GT

_CACHE: dict = {}

_SYNC_OK = {"InstEventSemaphore", "InstUnconditionalBranch", "InstHalt", "InstNoOp"}


def _split_waits(nc):
    """Move every instruction's on_wait onto same-engine NoOps placed just
    before it (this walrus rejects waits on most instruction formats)."""
    import concourse.mybir as mybir

    n = 0
    for func in nc.m.functions:
        for block in func.blocks:
            new = []
            for inst in block.instructions:
                si = getattr(inst, "sync_info", None)
                if si is not None and si.on_wait and type(inst).__name__ not in _SYNC_OK:
                    for w in si.on_wait:
                        n += 1
                        new.append(
                            mybir.InstNoOp(
                                name=f"W{n}_{inst.name}",
                                ins=[],
                                outs=[],
                                engine=inst.engine,
                                sync_info=mybir.SyncInfo(on_wait=[w], on_update=[]),
                            )
                        )
                    si.on_wait = []
                new.append(inst)
            block.instructions[:] = new
    return n


def _build(nb: int):
    from concourse import bass
    import concourse.mybir as mybir
    from concourse.tile import TileContext

    fp32 = mybir.dt.float32
    bf16 = mybir.dt.bfloat16
    i32 = mybir.dt.int32

    nc = bass.Bass(target_bir_lowering=False)

    x_t = nc.dram_tensor("x_t", [128, R], bf16, kind="ExternalInput")
    w1 = nc.dram_tensor("w1", [F, F], bf16, kind="ExternalInput")
    w2 = nc.dram_tensor("w2", [F, F], bf16, kind="ExternalInput")
    w3 = nc.dram_tensor("w3", [F, N_CLS], bf16, kind="ExternalInput")
    b12 = nc.dram_tensor("b12", [F, 2], fp32, kind="ExternalInput")
    b3 = nc.dram_tensor("b3", [N_CLS, 1], fp32, kind="ExternalInput")
    iota = nc.dram_tensor("iota", [128, 128], bf16, kind="ExternalInput")
    dinv = nc.dram_tensor("dinv", [128, T], fp32, kind="ExternalInput")
    gidx = nc.dram_tensor("gidx", [128, T, nb], i32, kind="ExternalInput")
    meta = nc.dram_tensor("meta", [128, T, 2, nb], bf16, kind="ExternalInput")

    out = nc.dram_tensor("out", [T, N_CLS, 128], fp32, kind="ExternalOutput")

    with (
        TileContext(nc) as tc,
        tc.tile_pool(name="const", bufs=1) as cpool,
        tc.tile_pool(name="ht", bufs=1) as hpool,
        tc.tile_pool(name="stag", bufs=1) as spool,
        tc.tile_pool(name="meta", bufs=2) as mpool,
        tc.tile_pool(name="gath", bufs=8) as gpool,
        tc.tile_pool(name="work", bufs=4) as wpool,
        tc.tile_pool(name="drain", bufs=3) as dpool,
        tc.tile_pool(name="psA", bufs=2, space="PSUM") as psA,
        tc.tile_pool(name="psB", bufs=4, space="PSUM") as psB,
        tc.tile_pool(name="dram", bufs=1, space="DRAM") as dram,
    ):
        w1_t = cpool.tile([F, F], bf16, name="w1_t")
        nc.sync.dma_start(w1_t[:], w1[:])
        w2_t = cpool.tile([F, F], bf16, name="w2_t")
        nc.sync.dma_start(w2_t[:], w2[:])
        w3_t = cpool.tile([F, N_CLS], bf16, name="w3_t")
        nc.sync.dma_start(w3_t[:], w3[:])
        b12_t = cpool.tile([F, 2], fp32, name="b12_t")
        nc.sync.dma_start(b12_t[:], b12[:])
        b3_t = cpool.tile([N_CLS, 1], fp32, name="b3_t")
        nc.sync.dma_start(b3_t[:], b3[:])
        iota_t = cpool.tile([128, 128], bf16, name="iota_t")
        nc.sync.dma_start(iota_t[:], iota[:])
        dinv_t = cpool.tile([128, T], fp32, name="dinv_t")
        nc.sync.dma_start(dinv_t[:], dinv[:])

        ht = hpool.tile([128, R], bf16, name="ht")
        nc.sync.dma_start(ht[:], x_t[:])

        ag_in = [
            dram.tile([128, T * F], bf16, name="ag_in1"),
            dram.tile([128, T * F], bf16, name="ag_in2"),
            dram.tile([128, T * N_CLS], bf16, name="ag_in3"),
        ]
        tables = [
            dram.tile([N_PAD, F], bf16, name="tbl1", addr_space="Shared"),
            dram.tile([N_PAD, F], bf16, name="tbl2", addr_space="Shared"),
            dram.tile([N_PAD, N_CLS], bf16, name="tbl3", addr_space="Shared"),
        ]

        for li, (w_t, fo) in enumerate([(w1_t, F), (w2_t, F), (w3_t, N_CLS)]):
            # ---- stage A ----
            stag = spool.tile([128, T * fo], bf16, tag="stag", name=f"stag{li}")
            for n in range(T):
                lm_p = psA.tile([128, fo], fp32, tag="lm", name=f"lm{li}_{n}")
                nc.tensor.matmul(
                    out=lm_p[:],
                    lhsT=ht[:, n * 128 : (n + 1) * 128],
                    rhs=w_t[:],
                    start=True,
                    stop=True,
                )
                nc.scalar.activation(
                    out=stag[:, n * fo : (n + 1) * fo],
                    in_=lm_p[:],
                    func=mybir.ActivationFunctionType.Copy,
                    bias=0.0,
                    scale=dinv_t[:, n : n + 1],
                )
            nc.sync.dma_start(ag_in[li][:], stag[:])
            nc.gpsimd.collective_compute(
                "AllGather",
                mybir.AluOpType.bypass,
                replica_groups=[list(range(N_CORES))],
                ins=[ag_in[li][:].opt()],
                outs=[tables[li][:].opt()],
            )

            # ---- stage B ----
            for g in range(NG):
                idx_g = mpool.tile([128, GT, nb], i32, tag="idx", name=f"idx{li}_{g}")
                nc.sync.dma_start(idx_g[:], gidx[:, g * GT : (g + 1) * GT, :])
                meta_g = mpool.tile([128, GT, 2, nb], bf16, tag="meta", name=f"meta{li}_{g}")
                nc.sync.dma_start(meta_g[:], meta[:, g * GT : (g + 1) * GT, :, :])

                for j in range(GT):
                    t = g * GT + j
                    agg_p = psB.tile([fo, 128], fp32, tag="agg", name=f"agg{li}_{t}")
                    for b in range(nb):
                        g_t = gpool.tile([128, fo], bf16, tag="g", name=f"g{li}_{t}_{b}")
                        nc.gpsimd.indirect_dma_start(
                            out=g_t[:],
                            out_offset=None,
                            in_=tables[li][:],
                            in_offset=bass.IndirectOffsetOnAxis(
                                ap=idx_g[:, j, b : b + 1], axis=0
                            ),
                        )
                        s_t = wpool.tile([128, 128], bf16, tag="s", name=f"s{li}_{t}_{b}")
                        nc.vector.scalar_tensor_tensor(
                            out=s_t[:],
                            in0=iota_t[:],
                            scalar=meta_g[:, j, 0, b : b + 1],
                            in1=meta_g[:, j, 1, b : b + 1].to_broadcast([128, 128]),
                            op0=mybir.AluOpType.is_equal,
                            op1=mybir.AluOpType.mult,
                        )
                        nc.tensor.matmul(
                            out=agg_p[:],
                            lhsT=g_t[:],
                            rhs=s_t[:],
                            start=(b == 0),
                            stop=(b == nb - 1),
                        )
                    if li < 2:
                        nc.scalar.activation(
                            out=ht[:, t * 128 : (t + 1) * 128],
                            in_=agg_p[:],
                            func=mybir.ActivationFunctionType.Relu,
                            bias=b12_t[:, li : li + 1],
                            scale=1.0,
                        )
                    else:
                        o_t = dpool.tile([N_CLS, 128], fp32, tag="o", name=f"o_{t}")
                        nc.scalar.activation(
                            out=o_t[:],
                            in_=agg_p[:],
                            func=mybir.ActivationFunctionType.Identity,
                            bias=b3_t[:, :1],
                            scale=1.0,
                        )
                        nc.sync.dma_start(out[t, :, :], o_t[:])

    _split_waits(nc)
    return nc


def _permute_ids(node):
    """Node id -> permuted table row id: pid = (c*128 + p)*T + t."""
    c, r = np.divmod(node, R)
    t, p_ = np.divmod(r, 128)
    return ((c * 128 + p_) * T + t).astype(np.int32)


def _preprocess(x, edge_index, edge_attr):
    N = x.shape[0]
    src = np.asarray(edge_index[0], np.int64)
    dst = np.asarray(edge_index[1], np.int64)
    w_e = np.asarray(edge_attr, np.float32)

    loop = np.arange(N, dtype=np.int64)
    src_a = np.concatenate([src, loop])
    dst_a = np.concatenate([dst, loop])
    w_a = np.concatenate([w_e, np.ones(N, np.float32)])

    deg = np.bincount(dst_a, weights=w_a.astype(np.float64), minlength=N)
    dinv = np.where(deg > 0, deg ** -0.5, 0.0).astype(np.float32)

    wgt = (w_a * dinv[dst_a]).astype(np.float32)

    order = np.argsort(dst_a, kind="stable")
    src_s = src_a[order]
    dst_s = dst_a[order]
    wgt_s = wgt[order]

    tile_id = (dst_s // 128).astype(np.int64)
    counts = np.bincount(tile_id, minlength=N_CORES * T)
    K = int(np.ceil(counts.max() / 128) * 128)
    nb = K // 128

    starts = np.zeros(N_CORES * T + 1, np.int64)
    np.cumsum(counts, out=starts[1:])
    offs = np.arange(len(src_s)) - starts[tile_id]

    idx_full = np.zeros((N_CORES * T, K), np.int32)
    slot_full = np.zeros((N_CORES * T, K), BF16)
    wgt_full = np.zeros((N_CORES * T, K), BF16)
    idx_full[tile_id, offs] = _permute_ids(src_s)
    slot_full[tile_id, offs] = (dst_s % 128).astype(np.float32)
    wgt_full[tile_id, offs] = wgt_s

    # [ct, K] -> [c][128, T, nb]; edge slot (b*128+p) of tile t -> (p, t, b)
    idx_r = idx_full.reshape(N_CORES, T, nb, 128).transpose(0, 3, 1, 2)
    slot_r = slot_full.reshape(N_CORES, T, nb, 128).transpose(0, 3, 1, 2)
    wgt_r = wgt_full.reshape(N_CORES, T, nb, 128).transpose(0, 3, 1, 2)
    meta_r = np.stack([slot_r, wgt_r], axis=3)  # [c][128, T, 2, nb]

    dinv_pad = np.zeros(N_PAD, np.float32)
    dinv_pad[:N] = dinv
    dinv_r = dinv_pad.reshape(N_CORES, T, 128).transpose(0, 2, 1)

    x_pad = np.zeros((N_PAD, F), np.float32)
    x_pad[:N] = np.asarray(x, np.float32)
    x_r = x_pad.reshape(N_CORES, R, F).transpose(0, 2, 1)  # [c][128, R]

    return nb, idx_r, meta_r, dinv_r, x_r


def kernel(x, edge_index, edge_attr, W1, b1, W2, b2, W3, b3):
    from concourse.bass_utils import run_bass_kernel_spmd

    t0 = time.perf_counter()
    nb, idx_r, meta_r, dinv_r, x_r = _preprocess(x, edge_index, edge_attr)

    iota_np = np.ascontiguousarray(
        np.broadcast_to(np.arange(128, dtype=np.float32), (128, 128))
    ).astype(BF16)
    consts = dict(
        w1=np.asarray(W1, np.float32).astype(BF16),
        w2=np.asarray(W2, np.float32).astype(BF16),
        w3=np.asarray(W3, np.float32).astype(BF16),
        b12=np.ascontiguousarray(
            np.stack([np.asarray(b1, np.float32), np.asarray(b2, np.float32)], axis=1)
        ),
        b3=np.ascontiguousarray(np.asarray(b3, np.float32).reshape(N_CLS, 1)),
        iota=iota_np,
    )
    in_maps = []
    for c in range(N_CORES):
        m = dict(consts)
        m["x_t"] = np.ascontiguousarray(x_r[c]).astype(BF16)
        m["dinv"] = np.ascontiguousarray(dinv_r[c])
        m["gidx"] = np.ascontiguousarray(idx_r[c])
        m["meta"] = np.ascontiguousarray(meta_r[c])
        in_maps.append(m)
    t1 = time.perf_counter()

    if nb not in _CACHE:
        _CACHE[nb] = _build(nb)
    nc = _CACHE[nb]
    t2 = time.perf_counter()

    res = run_bass_kernel_spmd(nc, in_maps, core_ids=list(range(N_CORES)))
    outs = res.results if hasattr(res, "results") else res
    t3 = time.perf_counter()

    parts = []
    for c in range(N_CORES):
        r = outs[c]
        a = np.asarray(r["out"] if isinstance(r, dict) else r, np.float32)
        parts.append(a.transpose(0, 2, 1).reshape(R, N_CLS))
    full = np.concatenate(parts, axis=0)[: x.shape[0]]
    t4 = time.perf_counter()
    print(
        f"[kernel] prep {t1-t0:.3f}s build {t2-t1:.3f}s run {t3-t2:.3f}s post {t4-t3:.3f}s",
        file=sys.stderr,
    )
    return full


# revision 3
# speedup vs baseline: 1.0049x; 1.0049x over previous
"""GCN (3-layer GCNConv) on 8 TRN2 NeuronCores — single-NEFF implementation.

Schedule per layer:
  stage A: table shard = dinv * (H @ W) in bf16 (PE matmul + ACT scale), staged
           in one SBUF block, one DMA to the AllGather bounce, AllGather ->
           full [N_PAD, Fo] bf16 table in every core's DRAM.
  stage B: per 128-dst-node tile: one indirect-DMA gather of K=128*NB source
           rows; per batch of 128 edges one DVE scalar_tensor_tensor builds
           S[e, slot] = wgt_e @ dstlocal_e ((iota == dstl) * wgt), TensorE
           accumulates PSUM[feat, slot] += G_b^T @ S_b in f32; ACT drains
           relu(psum + bias) with features on partitions (so the next layer's
           H @ W needs no transpose).
  norm = dinv[src]*w*dinv[dst] folds into the table rows (src side) and the
  edge weight stream (dst side).  Node ids are permuted
  (pid = (c*128+p)*T + t) so each core's SBUF-staged shard all-gathers into a
  row-gatherable table.

Toolchain quirk: this walrus build allows ~zero semaphore waits on most
compute-instruction formats ("Too many sync wait commands").  After Tile
schedules the program, _split_waits() moves every on_wait onto its own
same-engine InstNoOp inserted immediately before the instruction — semantics
identical, and NoOps accept waits.
"""
import sys
import time

import numpy as np
import ml_dtypes

for p in ("/opt/trn_rl_repo",):
    if p not in sys.path:
        sys.path.insert(0, p)

BF16 = ml_dtypes.bfloat16
N_CORES = 8
R = 12544                 # dst rows per core
N_PAD = N_CORES * R       # 100352
T = R // 128              # 98 dst tiles per core
F = 128
N_CLS = 64
GT = 7                    # meta-load group: 7 tiles; 98 = 14*7
NG = T // GT

_CACHE: dict = {}

_SYNC_OK = {"InstEventSemaphore", "InstUnconditionalBranch", "InstHalt", "InstNoOp"}


def _split_waits(nc):
    """Move every instruction's on_wait onto same-engine NoOps placed just
    before it (this walrus rejects waits on most instruction formats)."""
    import concourse.mybir as mybir

    n = 0
    for func in nc.m.functions:
        for block in func.blocks:
            new = []
            for inst in block.instructions:
                si = getattr(inst, "sync_info", None)
                if si is not None and si.on_wait and type(inst).__name__ not in _SYNC_OK:
                    for w in si.on_wait:
                        n += 1
                        new.append(
                            mybir.InstNoOp(
                                name=f"W{n}_{inst.name}",
                                ins=[],
                                outs=[],
                                engine=inst.engine,
                                sync_info=mybir.SyncInfo(on_wait=[w], on_update=[]),
                            )
                        )
                    si.on_wait = []
                new.append(inst)
            block.instructions[:] = new
    return n


def _build(nb: int):
    from concourse import bass
    import concourse.mybir as mybir
    from concourse.tile import TileContext

    fp32 = mybir.dt.float32
    bf16 = mybir.dt.bfloat16
    i32 = mybir.dt.int32

    nc = bass.Bass(target_bir_lowering=False)

    x_t = nc.dram_tensor("x_t", [128, R], bf16, kind="ExternalInput")
    w1 = nc.dram_tensor("w1", [F, F], bf16, kind="ExternalInput")
    w2 = nc.dram_tensor("w2", [F, F], bf16, kind="ExternalInput")
    w3 = nc.dram_tensor("w3", [F, N_CLS], bf16, kind="ExternalInput")
    b12 = nc.dram_tensor("b12", [F, 2], fp32, kind="ExternalInput")
    b3 = nc.dram_tensor("b3", [N_CLS, 1], fp32, kind="ExternalInput")
    iota = nc.dram_tensor("iota", [128, 128], bf16, kind="ExternalInput")
    dinv = nc.dram_tensor("dinv", [128, T], fp32, kind="ExternalInput")
    gidx = nc.dram_tensor("gidx", [128, T, nb], i32, kind="ExternalInput")
    meta = nc.dram_tensor("meta", [128, T, 2, nb], bf16, kind="ExternalInput")

    out = nc.dram_tensor("out", [T, N_CLS, 128], bf16, kind="ExternalOutput")

    with (
        TileContext(nc) as tc,
        tc.tile_pool(name="const", bufs=1) as cpool,
        tc.tile_pool(name="ht", bufs=1) as hpool,
        tc.tile_pool(name="stag", bufs=1) as spool,
        tc.tile_pool(name="meta", bufs=2) as mpool,
        tc.tile_pool(name="gath", bufs=8) as gpool,
        tc.tile_pool(name="work", bufs=4) as wpool,
        tc.tile_pool(name="drain", bufs=3) as dpool,
        tc.tile_pool(name="psA", bufs=2, space="PSUM") as psA,
        tc.tile_pool(name="psB", bufs=4, space="PSUM") as psB,
        tc.tile_pool(name="dram", bufs=1, space="DRAM") as dram,
    ):
        w1_t = cpool.tile([F, F], bf16, name="w1_t")
        nc.sync.dma_start(w1_t[:], w1[:])
        w2_t = cpool.tile([F, F], bf16, name="w2_t")
        nc.sync.dma_start(w2_t[:], w2[:])
        w3_t = cpool.tile([F, N_CLS], bf16, name="w3_t")
        nc.sync.dma_start(w3_t[:], w3[:])
        b12_t = cpool.tile([F, 2], fp32, name="b12_t")
        nc.sync.dma_start(b12_t[:], b12[:])
        b3_t = cpool.tile([N_CLS, 1], fp32, name="b3_t")
        nc.sync.dma_start(b3_t[:], b3[:])
        iota_t = cpool.tile([128, 128], bf16, name="iota_t")
        nc.sync.dma_start(iota_t[:], iota[:])
        dinv_t = cpool.tile([128, T], fp32, name="dinv_t")
        nc.sync.dma_start(dinv_t[:], dinv[:])

        ht = hpool.tile([128, R], bf16, name="ht")
        nc.sync.dma_start(ht[:], x_t[:])

        ag_in = [
            dram.tile([128, T * F], bf16, name="ag_in1"),
            dram.tile([128, T * F], bf16, name="ag_in2"),
            dram.tile([128, T * N_CLS], bf16, name="ag_in3"),
        ]
        tables = [
            dram.tile([N_PAD, F], bf16, name="tbl1", addr_space="Shared"),
            dram.tile([N_PAD, F], bf16, name="tbl2", addr_space="Shared"),
            dram.tile([N_PAD, N_CLS], bf16, name="tbl3", addr_space="Shared"),
        ]

        for li, (w_t, fo) in enumerate([(w1_t, F), (w2_t, F), (w3_t, N_CLS)]):
            # ---- stage A ----
            stag = spool.tile([128, T * fo], bf16, tag="stag", name=f"stag{li}")
            for n in range(T):
                lm_p = psA.tile([128, fo], fp32, tag="lm", name=f"lm{li}_{n}")
                nc.tensor.matmul(
                    out=lm_p[:],
                    lhsT=ht[:, n * 128 : (n + 1) * 128],
                    rhs=w_t[:],
                    start=True,
                    stop=True,
                )
                nc.scalar.activation(
                    out=stag[:, n * fo : (n + 1) * fo],
                    in_=lm_p[:],
                    func=mybir.ActivationFunctionType.Copy,
                    bias=0.0,
                    scale=dinv_t[:, n : n + 1],
                )
            nc.sync.dma_start(ag_in[li][:], stag[:])
            nc.gpsimd.collective_compute(
                "AllGather",
                mybir.AluOpType.bypass,
                replica_groups=[list(range(N_CORES))],
                ins=[ag_in[li][:].opt()],
                outs=[tables[li][:].opt()],
            )

            # ---- stage B ----
            for g in range(NG):
                idx_g = mpool.tile([128, GT, nb], i32, tag="idx", name=f"idx{li}_{g}")
                nc.sync.dma_start(idx_g[:], gidx[:, g * GT : (g + 1) * GT, :])
                meta_g = mpool.tile([128, GT, 2, nb], bf16, tag="meta", name=f"meta{li}_{g}")
                nc.sync.dma_start(meta_g[:], meta[:, g * GT : (g + 1) * GT, :, :])

                for j in range(GT):
                    t = g * GT + j
                    agg_p = psB.tile([fo, 128], fp32, tag="agg", name=f"agg{li}_{t}")
                    for b in range(nb):
                        g_t = gpool.tile([128, fo], bf16, tag="g", name=f"g{li}_{t}_{b}")
                        nc.gpsimd.indirect_dma_start(
                            out=g_t[:],
                            out_offset=None,
                            in_=tables[li][:],
                            in_offset=bass.IndirectOffsetOnAxis(
                                ap=idx_g[:, j, b : b + 1], axis=0
                            ),
                        )
                        s_t = wpool.tile([128, 128], bf16, tag="s", name=f"s{li}_{t}_{b}")
                        nc.vector.scalar_tensor_tensor(
                            out=s_t[:],
                            in0=iota_t[:],
                            scalar=meta_g[:, j, 0, b : b + 1],
                            in1=meta_g[:, j, 1, b : b + 1].to_broadcast([128, 128]),
                            op0=mybir.AluOpType.is_equal,
                            op1=mybir.AluOpType.mult,
                        )
                        nc.tensor.matmul(
                            out=agg_p[:],
                            lhsT=g_t[:],
                            rhs=s_t[:],
                            start=(b == 0),
                            stop=(b == nb - 1),
                        )
                    if li < 2:
                        nc.scalar.activation(
                            out=ht[:, t * 128 : (t + 1) * 128],
                            in_=agg_p[:],
                            func=mybir.ActivationFunctionType.Relu,
                            bias=b12_t[:, li : li + 1],
                            scale=1.0,
                        )
                    else:
                        o_t = dpool.tile([N_CLS, 128], bf16, tag="o", name=f"o_{t}")
                        nc.scalar.activation(
                            out=o_t[:],
                            in_=agg_p[:],
                            func=mybir.ActivationFunctionType.Identity,
                            bias=b3_t[:, :1],
                            scale=1.0,
                        )
                        nc.sync.dma_start(out[t, :, :], o_t[:])

    _split_waits(nc)
    return nc


def _permute_ids(node):
    """Node id -> permuted table row id: pid = (c*128 + p)*T + t."""
    c, r = np.divmod(node, R)
    t, p_ = np.divmod(r, 128)
    return ((c * 128 + p_) * T + t).astype(np.int32)


def _preprocess(x, edge_index, edge_attr):
    N = x.shape[0]
    src = np.asarray(edge_index[0], np.int64)
    dst = np.asarray(edge_index[1], np.int64)
    w_e = np.asarray(edge_attr, np.float32)

    loop = np.arange(N, dtype=np.int64)
    src_a = np.concatenate([src, loop])
    dst_a = np.concatenate([dst, loop])
    w_a = np.concatenate([w_e, np.ones(N, np.float32)])

    deg = np.bincount(dst_a, weights=w_a.astype(np.float64), minlength=N)
    dinv = np.where(deg > 0, deg ** -0.5, 0.0).astype(np.float32)

    wgt = (w_a * dinv[dst_a]).astype(np.float32)

    order = np.argsort(dst_a.astype(np.int32), kind="stable")
    src_s = src_a[order]
    dst_s = dst_a[order]
    wgt_s = wgt[order]

    tile_id = (dst_s // 128).astype(np.int64)
    counts = np.bincount(tile_id, minlength=N_CORES * T)
    K = int(np.ceil(counts.max() / 128) * 128)
    nb = K // 128

    starts = np.zeros(N_CORES * T + 1, np.int64)
    np.cumsum(counts, out=starts[1:])
    offs = np.arange(len(src_s)) - starts[tile_id]

    idx_full = np.zeros((N_CORES * T, K), np.int32)
    slot_full = np.zeros((N_CORES * T, K), BF16)
    wgt_full = np.zeros((N_CORES * T, K), BF16)
    idx_full[tile_id, offs] = _permute_ids(src_s)
    slot_full[tile_id, offs] = (dst_s % 128).astype(np.float32)
    wgt_full[tile_id, offs] = wgt_s

    # [ct, K] -> [c][128, T, nb]; edge slot (b*128+p) of tile t -> (p, t, b)
    idx_r = idx_full.reshape(N_CORES, T, nb, 128).transpose(0, 3, 1, 2)
    slot_r = slot_full.reshape(N_CORES, T, nb, 128).transpose(0, 3, 1, 2)
    wgt_r = wgt_full.reshape(N_CORES, T, nb, 128).transpose(0, 3, 1, 2)
    meta_r = np.stack([slot_r, wgt_r], axis=3)  # [c][128, T, 2, nb]

    dinv_pad = np.zeros(N_PAD, np.float32)
    dinv_pad[:N] = dinv
    dinv_r = dinv_pad.reshape(N_CORES, T, 128).transpose(0, 2, 1)

    x_pad = np.zeros((N_PAD, F), BF16)
    x_pad[:N] = np.asarray(x, np.float32).astype(BF16)
    x_r = x_pad.reshape(N_CORES, R, F).transpose(0, 2, 1)  # [c][128, R]

    return nb, idx_r, meta_r, dinv_r, x_r


def kernel(x, edge_index, edge_attr, W1, b1, W2, b2, W3, b3):
    from concourse.bass_utils import run_bass_kernel_spmd

    t0 = time.perf_counter()
    nb, idx_r, meta_r, dinv_r, x_r = _preprocess(x, edge_index, edge_attr)

    iota_np = np.ascontiguousarray(
        np.broadcast_to(np.arange(128, dtype=np.float32), (128, 128))
    ).astype(BF16)
    consts = dict(
        w1=np.asarray(W1, np.float32).astype(BF16),
        w2=np.asarray(W2, np.float32).astype(BF16),
        w3=np.asarray(W3, np.float32).astype(BF16),
        b12=np.ascontiguousarray(
            np.stack([np.asarray(b1, np.float32), np.asarray(b2, np.float32)], axis=1)
        ),
        b3=np.ascontiguousarray(np.asarray(b3, np.float32).reshape(N_CLS, 1)),
        iota=iota_np,
    )
    in_maps = []
    for c in range(N_CORES):
        m = dict(consts)
        m["x_t"] = np.ascontiguousarray(x_r[c])
        m["dinv"] = np.ascontiguousarray(dinv_r[c])
        m["gidx"] = np.ascontiguousarray(idx_r[c])
        m["meta"] = np.ascontiguousarray(meta_r[c])
        in_maps.append(m)
    t1 = time.perf_counter()

    if nb not in _CACHE:
        _CACHE[nb] = _build(nb)
    nc = _CACHE[nb]
    t2 = time.perf_counter()

    res = run_bass_kernel_spmd(nc, in_maps, core_ids=list(range(N_CORES)))
    outs = res.results if hasattr(res, "results") else res
    t3 = time.perf_counter()

    parts = []
    for c in range(N_CORES):
        r = outs[c]
        a = np.asarray(r["out"] if isinstance(r, dict) else r, np.float32)
        parts.append(a.transpose(0, 2, 1).reshape(R, N_CLS))
    full = np.concatenate(parts, axis=0)[: x.shape[0]]
    t4 = time.perf_counter()
    print(
        f"[kernel] prep {t1-t0:.3f}s build {t2-t1:.3f}s run {t3-t2:.3f}s post {t4-t3:.3f}s",
        file=sys.stderr,
    )
    return full


# revision 4
# speedup vs baseline: 1.2049x; 1.1990x over previous
"""GCN (3-layer GCNConv) on 8 TRN2 NeuronCores — single-NEFF implementation.

Schedule per layer:
  stage A: table shard = dinv * (H @ W) in bf16 (PE matmul + ACT scale), staged
           in one SBUF block, one DMA to the AllGather bounce, AllGather ->
           full [N_PAD, Fo] bf16 table in every core's DRAM.
  stage B: per 128-dst-node tile: one indirect-DMA gather of K=128*NB source
           rows; per batch of 128 edges one DVE scalar_tensor_tensor builds
           S[e, slot] = wgt_e @ dstlocal_e ((iota == dstl) * wgt), TensorE
           accumulates PSUM[feat, slot] += G_b^T @ S_b in f32; ACT drains
           relu(psum + bias) with features on partitions (so the next layer's
           H @ W needs no transpose).
  norm = dinv[src]*w*dinv[dst] folds into the table rows (src side) and the
  edge weight stream (dst side).  Node ids are permuted
  (pid = (c*128+p)*T + t) so each core's SBUF-staged shard all-gathers into a
  row-gatherable table.

Toolchain quirk: this walrus build allows ~zero semaphore waits on most
compute-instruction formats ("Too many sync wait commands").  After Tile
schedules the program, _split_waits() moves every on_wait onto its own
same-engine InstNoOp inserted immediately before the instruction — semantics
identical, and NoOps accept waits.
"""
import sys
import time

import numpy as np
import ml_dtypes

for p in ("/opt/trn_rl_repo",):
    if p not in sys.path:
        sys.path.insert(0, p)

BF16 = ml_dtypes.bfloat16
N_CORES = 8
R = 12544                 # dst rows per core
N_PAD = N_CORES * R       # 100352
T = R // 128              # 98 dst tiles per core
F = 128
N_CLS = 64
GT = 7                    # meta-load group: 7 tiles; 98 = 14*7
NG = T // GT

_CACHE: dict = {}

_SYNC_OK = {"InstEventSemaphore", "InstUnconditionalBranch", "InstHalt", "InstNoOp"}


def _split_waits(nc):
    """Move every instruction's on_wait onto same-engine NoOps placed just
    before it (this walrus rejects waits on most instruction formats).

    Additionally thin the DMASW waits Tile puts on consecutive Pool-engine
    indirect gathers: they serialize each gather behind the previous one's
    full DMA completion.  The gathers are independent (distinct slots, WAR
    already guarded by the PE wait), so keep only every 4th DMASW wait as a
    SWDGE descriptor-ring capacity bound (cumulative thresholds make the kept
    wait cover all earlier gathers)."""
    import concourse.mybir as mybir
    from concourse.indirect_dma import is_vector_indirect_dma_ap

    n = 0
    pool = mybir.EngineType.Pool
    for func in nc.m.functions:
        for block in func.blocks:
            new = []
            gather_i = 0
            for inst in block.instructions:
                si = getattr(inst, "sync_info", None)
                is_gather = (
                    isinstance(inst, mybir.InstDMACopy)
                    and inst.engine == pool
                    and is_vector_indirect_dma_ap(
                        [a for a in inst.ins if not isinstance(a, mybir.RegisterAccess)]
                    )
                )
                if si is not None and si.on_wait and type(inst).__name__ not in _SYNC_OK:
                    for w in si.on_wait:
                        if (
                            is_gather
                            and w.ant_name.startswith("DMASW")
                            and gather_i % 4 != 0
                        ):
                            continue
                        n += 1
                        new.append(
                            mybir.InstNoOp(
                                name=f"W{n}_{inst.name}",
                                ins=[],
                                outs=[],
                                engine=inst.engine,
                                sync_info=mybir.SyncInfo(on_wait=[w], on_update=[]),
                            )
                        )
                    si.on_wait = []
                if is_gather:
                    gather_i += 1
                new.append(inst)
            block.instructions[:] = new
    return n


def _build(nb: int):
    from concourse import bass
    import concourse.mybir as mybir
    from concourse.tile import TileContext

    fp32 = mybir.dt.float32
    bf16 = mybir.dt.bfloat16
    i32 = mybir.dt.int32

    nc = bass.Bass(target_bir_lowering=False)

    x_t = nc.dram_tensor("x_t", [128, R], bf16, kind="ExternalInput")
    w1 = nc.dram_tensor("w1", [F, F], bf16, kind="ExternalInput")
    w2 = nc.dram_tensor("w2", [F, F], bf16, kind="ExternalInput")
    w3 = nc.dram_tensor("w3", [F, N_CLS], bf16, kind="ExternalInput")
    b12 = nc.dram_tensor("b12", [F, 2], fp32, kind="ExternalInput")
    b3 = nc.dram_tensor("b3", [N_CLS, 1], fp32, kind="ExternalInput")
    iota = nc.dram_tensor("iota", [128, 128], bf16, kind="ExternalInput")
    dinv = nc.dram_tensor("dinv", [128, T], fp32, kind="ExternalInput")
    gidx = nc.dram_tensor("gidx", [128, T, nb], i32, kind="ExternalInput")
    meta = nc.dram_tensor("meta", [128, T, 2, nb], bf16, kind="ExternalInput")

    out = nc.dram_tensor("out", [T, N_CLS, 128], bf16, kind="ExternalOutput")

    with (
        TileContext(nc) as tc,
        tc.tile_pool(name="const", bufs=1) as cpool,
        tc.tile_pool(name="ht", bufs=1) as hpool,
        tc.tile_pool(name="stag", bufs=1) as spool,
        tc.tile_pool(name="meta", bufs=2) as mpool,
        tc.tile_pool(name="gath", bufs=8) as gpool,
        tc.tile_pool(name="work", bufs=4) as wpool,
        tc.tile_pool(name="drain", bufs=3) as dpool,
        tc.tile_pool(name="psA", bufs=2, space="PSUM") as psA,
        tc.tile_pool(name="psB", bufs=4, space="PSUM") as psB,
        tc.tile_pool(name="dram", bufs=1, space="DRAM") as dram,
    ):
        w1_t = cpool.tile([F, F], bf16, name="w1_t")
        nc.sync.dma_start(w1_t[:], w1[:])
        w2_t = cpool.tile([F, F], bf16, name="w2_t")
        nc.sync.dma_start(w2_t[:], w2[:])
        w3_t = cpool.tile([F, N_CLS], bf16, name="w3_t")
        nc.sync.dma_start(w3_t[:], w3[:])
        b12_t = cpool.tile([F, 2], fp32, name="b12_t")
        nc.sync.dma_start(b12_t[:], b12[:])
        b3_t = cpool.tile([N_CLS, 1], fp32, name="b3_t")
        nc.sync.dma_start(b3_t[:], b3[:])
        iota_t = cpool.tile([128, 128], bf16, name="iota_t")
        nc.sync.dma_start(iota_t[:], iota[:])
        dinv_t = cpool.tile([128, T], fp32, name="dinv_t")
        nc.sync.dma_start(dinv_t[:], dinv[:])

        ht = hpool.tile([128, R], bf16, name="ht")
        nc.sync.dma_start(ht[:], x_t[:])

        ag_in = [
            dram.tile([128, T * F], bf16, name="ag_in1"),
            dram.tile([128, T * F], bf16, name="ag_in2"),
            dram.tile([128, T * N_CLS], bf16, name="ag_in3"),
        ]
        tables = [
            dram.tile([N_PAD, F], bf16, name="tbl1", addr_space="Shared"),
            dram.tile([N_PAD, F], bf16, name="tbl2", addr_space="Shared"),
            dram.tile([N_PAD, N_CLS], bf16, name="tbl3", addr_space="Shared"),
        ]

        for li, (w_t, fo) in enumerate([(w1_t, F), (w2_t, F), (w3_t, N_CLS)]):
            # ---- stage A ----
            stag = spool.tile([128, T * fo], bf16, tag="stag", name=f"stag{li}")
            for n in range(T):
                lm_p = psA.tile([128, fo], fp32, tag="lm", name=f"lm{li}_{n}")
                nc.tensor.matmul(
                    out=lm_p[:],
                    lhsT=ht[:, n * 128 : (n + 1) * 128],
                    rhs=w_t[:],
                    start=True,
                    stop=True,
                )
                nc.scalar.activation(
                    out=stag[:, n * fo : (n + 1) * fo],
                    in_=lm_p[:],
                    func=mybir.ActivationFunctionType.Copy,
                    bias=0.0,
                    scale=dinv_t[:, n : n + 1],
                )
            nc.sync.dma_start(ag_in[li][:], stag[:])
            nc.gpsimd.collective_compute(
                "AllGather",
                mybir.AluOpType.bypass,
                replica_groups=[list(range(N_CORES))],
                ins=[ag_in[li][:].opt()],
                outs=[tables[li][:].opt()],
            )

            # ---- stage B ----
            for g in range(NG):
                idx_g = mpool.tile([128, GT, nb], i32, tag="idx", name=f"idx{li}_{g}")
                nc.sync.dma_start(idx_g[:], gidx[:, g * GT : (g + 1) * GT, :])
                meta_g = mpool.tile([128, GT, 2, nb], bf16, tag="meta", name=f"meta{li}_{g}")
                nc.sync.dma_start(meta_g[:], meta[:, g * GT : (g + 1) * GT, :, :])

                for j in range(GT):
                    t = g * GT + j
                    agg_p = psB.tile([fo, 128], fp32, tag="agg", name=f"agg{li}_{t}")
                    for b in range(nb):
                        g_t = gpool.tile([128, fo], bf16, tag="g", name=f"g{li}_{t}_{b}")
                        nc.gpsimd.indirect_dma_start(
                            out=g_t[:],
                            out_offset=None,
                            in_=tables[li][:],
                            in_offset=bass.IndirectOffsetOnAxis(
                                ap=idx_g[:, j, b : b + 1], axis=0
                            ),
                        )
                        s_t = wpool.tile([128, 128], bf16, tag="s", name=f"s{li}_{t}_{b}")
                        nc.vector.scalar_tensor_tensor(
                            out=s_t[:],
                            in0=iota_t[:],
                            scalar=meta_g[:, j, 0, b : b + 1],
                            in1=meta_g[:, j, 1, b : b + 1].to_broadcast([128, 128]),
                            op0=mybir.AluOpType.is_equal,
                            op1=mybir.AluOpType.mult,
                        )
                        nc.tensor.matmul(
                            out=agg_p[:],
                            lhsT=g_t[:],
                            rhs=s_t[:],
                            start=(b == 0),
                            stop=(b == nb - 1),
                        )
                    if li < 2:
                        nc.scalar.activation(
                            out=ht[:, t * 128 : (t + 1) * 128],
                            in_=agg_p[:],
                            func=mybir.ActivationFunctionType.Relu,
                            bias=b12_t[:, li : li + 1],
                            scale=1.0,
                        )
                    else:
                        o_t = dpool.tile([N_CLS, 128], bf16, tag="o", name=f"o_{t}")
                        nc.scalar.activation(
                            out=o_t[:],
                            in_=agg_p[:],
                            func=mybir.ActivationFunctionType.Identity,
                            bias=b3_t[:, :1],
                            scale=1.0,
                        )
                        nc.sync.dma_start(out[t, :, :], o_t[:])

    _split_waits(nc)
    return nc


def _permute_ids(node):
    """Node id -> permuted table row id: pid = (c*128 + p)*T + t."""
    c, r = np.divmod(node, R)
    t, p_ = np.divmod(r, 128)
    return ((c * 128 + p_) * T + t).astype(np.int32)


def _preprocess(x, edge_index, edge_attr):
    N = x.shape[0]
    src = np.asarray(edge_index[0], np.int64)
    dst = np.asarray(edge_index[1], np.int64)
    w_e = np.asarray(edge_attr, np.float32)

    loop = np.arange(N, dtype=np.int64)
    src_a = np.concatenate([src, loop])
    dst_a = np.concatenate([dst, loop])
    w_a = np.concatenate([w_e, np.ones(N, np.float32)])

    deg = np.bincount(dst_a, weights=w_a.astype(np.float64), minlength=N)
    dinv = np.where(deg > 0, deg ** -0.5, 0.0).astype(np.float32)

    wgt = (w_a * dinv[dst_a]).astype(np.float32)

    order = np.argsort(dst_a.astype(np.int32), kind="stable")
    src_s = src_a[order]
    dst_s = dst_a[order]
    wgt_s = wgt[order]

    tile_id = (dst_s // 128).astype(np.int64)
    counts = np.bincount(tile_id, minlength=N_CORES * T)
    K = int(np.ceil(counts.max() / 128) * 128)
    nb = K // 128

    starts = np.zeros(N_CORES * T + 1, np.int64)
    np.cumsum(counts, out=starts[1:])
    offs = np.arange(len(src_s)) - starts[tile_id]

    idx_full = np.zeros((N_CORES * T, K), np.int32)
    slot_full = np.zeros((N_CORES * T, K), BF16)
    wgt_full = np.zeros((N_CORES * T, K), BF16)
    idx_full[tile_id, offs] = _permute_ids(src_s)
    slot_full[tile_id, offs] = (dst_s % 128).astype(np.float32)
    wgt_full[tile_id, offs] = wgt_s

    # [ct, K] -> [c][128, T, nb]; edge slot (b*128+p) of tile t -> (p, t, b)
    idx_r = idx_full.reshape(N_CORES, T, nb, 128).transpose(0, 3, 1, 2)
    slot_r = slot_full.reshape(N_CORES, T, nb, 128).transpose(0, 3, 1, 2)
    wgt_r = wgt_full.reshape(N_CORES, T, nb, 128).transpose(0, 3, 1, 2)
    meta_r = np.stack([slot_r, wgt_r], axis=3)  # [c][128, T, 2, nb]

    dinv_pad = np.zeros(N_PAD, np.float32)
    dinv_pad[:N] = dinv
    dinv_r = dinv_pad.reshape(N_CORES, T, 128).transpose(0, 2, 1)

    x_pad = np.zeros((N_PAD, F), BF16)
    x_pad[:N] = np.asarray(x, np.float32).astype(BF16)
    x_r = x_pad.reshape(N_CORES, R, F).transpose(0, 2, 1)  # [c][128, R]

    return nb, idx_r, meta_r, dinv_r, x_r


def kernel(x, edge_index, edge_attr, W1, b1, W2, b2, W3, b3):
    from concourse.bass_utils import run_bass_kernel_spmd

    t0 = time.perf_counter()
    nb, idx_r, meta_r, dinv_r, x_r = _preprocess(x, edge_index, edge_attr)

    iota_np = np.ascontiguousarray(
        np.broadcast_to(np.arange(128, dtype=np.float32), (128, 128))
    ).astype(BF16)
    consts = dict(
        w1=np.asarray(W1, np.float32).astype(BF16),
        w2=np.asarray(W2, np.float32).astype(BF16),
        w3=np.asarray(W3, np.float32).astype(BF16),
        b12=np.ascontiguousarray(
            np.stack([np.asarray(b1, np.float32), np.asarray(b2, np.float32)], axis=1)
        ),
        b3=np.ascontiguousarray(np.asarray(b3, np.float32).reshape(N_CLS, 1)),
        iota=iota_np,
    )
    in_maps = []
    for c in range(N_CORES):
        m = dict(consts)
        m["x_t"] = np.ascontiguousarray(x_r[c])
        m["dinv"] = np.ascontiguousarray(dinv_r[c])
        m["gidx"] = np.ascontiguousarray(idx_r[c])
        m["meta"] = np.ascontiguousarray(meta_r[c])
        in_maps.append(m)
    t1 = time.perf_counter()

    if nb not in _CACHE:
        _CACHE[nb] = _build(nb)
    nc = _CACHE[nb]
    t2 = time.perf_counter()

    res = run_bass_kernel_spmd(nc, in_maps, core_ids=list(range(N_CORES)))
    outs = res.results if hasattr(res, "results") else res
    t3 = time.perf_counter()

    parts = []
    for c in range(N_CORES):
        r = outs[c]
        a = np.asarray(r["out"] if isinstance(r, dict) else r, np.float32)
        parts.append(a.transpose(0, 2, 1).reshape(R, N_CLS))
    full = np.concatenate(parts, axis=0)[: x.shape[0]]
    t4 = time.perf_counter()
    print(
        f"[kernel] prep {t1-t0:.3f}s build {t2-t1:.3f}s run {t3-t2:.3f}s post {t4-t3:.3f}s",
        file=sys.stderr,
    )
    return full
